# revision 23
# baseline (speedup 1.0000x reference)
"""AMICO ADMM solver on 8 Trainium2 NeuronCores.

Problem: X = argmin ||Y^T - A x||^2 + lam*||x||_1 s.t. x >= 0, solved with
max_iter ADMM steps (rho=1, lam=0.1) exactly as in the reference scan.

Algebraic reduction (tracking only v = x + u):
    v_1 = G                      with G  = Minv @ A^T @ Y^T
    for i = 2..N:
        w   = |v - t|            (t = lam/rho)
        S   = min(v, t) + Gb     (Gb = G - t * Minv @ 1)
        v'  = Minv @ w + S
    output x_N = Minv @ w_{N-1} + Gb

since z = relu(v - t), u' = v - z = min(v, t), and z - u' = |v - t| - t.

Sharding: data-parallel over voxels (B=4096 -> 512 per core); A-derived
matrices (Minv, Ht) replicated; no cross-core communication.

Implementation notes (measured on silicon):
 - All matmul operands are fp16 (fp8/bf16 lose too much accuracy over 30
   non-converged iterations; verified by simulation).
 - In-place S-fold: after each iteration's 4-matmul accumulation group
   fills PSUM bank m with v, ACT reads it for w = |v - t| and then DVE
   rewrites the bank IN PLACE with S' = min(v, t) + Gb.  The next
   iteration's matmuls accumulate onto S' with start=False - the PSUM
   has_written bits persist from the previous matmul group (only
   first_mm=1 clears them), so no identity matmuls are needed.  This
   cuts PE work from 18 to 16 matmuls/iteration and removes the
   separate fp16 v materialization from the DVE.
 - PE warm-up: ~8 dummy matmuls on a scratch PSUM bank issue during the
   input-DMA wait so the HAM clock gate reaches 2.4 GHz before the real
   matmuls start.
 - Input DMAs issue in parallel from the sync, scalar(ACT) and gpsimd
   queues; output DMAs are split across sync/gpsimd, per chunk, so the
   final transfers overlap the last iteration's compute.
"""

import numpy as np

B_VOX = 4096
M_MEAS = 256
K_ATOMS = 512
P = 128
N_CORES = 8
BS = B_VOX // N_CORES  # 512 voxels per core
KB = K_ATOMS // P  # 4 chunks of the contraction/output dim
LAM = 0.1
RHO = 1.0
THR = LAM / RHO

_NC_CACHE = {}

# packed layout offsets (fp16 elements per partition row)
O_HT0 = 0
O_YT0 = 512
O_HY1 = 1024            # Ht1 | Yt1
O_CN = 2048             # cneg [KB]
O_MI = 2052             # Minv, KB chunks of 512
NPACK = O_MI + KB * K_ATOMS  # 4100


def _build(niter):
    import concourse.mybir as mybir
    import concourse.tile as tile
    from concourse import bacc

    f32 = mybir.dt.float32
    f16 = mybir.dt.float16
    Alu = mybir.AluOpType
    Act = mybir.ActivationFunctionType

    nc = bacc.Bacc(None, target_bir_lowering=False)
    packed = nc.declare_dram_parameter("packed", [P, NPACK], f16, isOutput=False)
    out = nc.declare_dram_parameter("out", [K_ATOMS, BS], f16, isOutput=True)

    with tile.TileContext(nc) as tc:
        with (
            tc.tile_pool(name="const", bufs=1) as cpool,
            tc.tile_pool(name="w", bufs=8) as wpool,
            tc.tile_pool(name="o", bufs=4) as opool,
            tc.tile_pool(name="psum", bufs=1, space="PSUM") as ppool,
            tc.tile_pool(name="pwarm", bufs=1, space="PSUM") as ppwarm,
        ):
            # ---- PE warm-up: dummy matmuls on uninitialized scratch ----
            # (no data deps, so they issue right after the preamble barrier
            # and keep the HAM activity window busy during the DMA wait)
            sc_w = cpool.tile([P, P], f16)
            sc_r = cpool.tile([P, BS], f16)
            pwarm = ppwarm.tile([P, BS], f32)
            nc.vector.memset(sc_w[:], 0.0)
            nc.vector.memset(sc_r[:], 0.0)
            for _ in range(8):
                nc.tensor.matmul(pwarm[:], lhsT=sc_w[:], rhs=sc_r[:],
                                 start=True, stop=True)

            nb = cpool.tile([P, 1], f32)
            nc.vector.memset(nb[:], -THR)

            # ---- input DMAs: parallel issue across sync/scalar/gpsimd ----
            hy_sb = cpool.tile([P, 2 * (K_ATOMS + BS)], f16)
            mi_sb = cpool.tile([P, KB + KB * K_ATOMS], f16)  # cneg + Minv
            nc.sync.dma_start(mi_sb[:, 0:KB], packed[:, O_CN : O_CN + KB])
            nc.sync.dma_start(hy_sb[:, 0:512], packed[:, O_HT0:O_YT0])
            nc.scalar.dma_start(hy_sb[:, 512:1024], packed[:, O_YT0:O_HY1])
            nc.sync.dma_start(hy_sb[:, 1024:1536], packed[:, O_HY1 : O_HY1 + 512])
            nc.scalar.dma_start(hy_sb[:, 1536:2048],
                                packed[:, O_HY1 + 512 : O_CN])
            nc.sync.dma_start(mi_sb[:, KB : KB + 512],
                              packed[:, O_CN + KB : O_CN + KB + 512])
            nc.scalar.dma_start(mi_sb[:, KB + 512 : KB + 1024],
                                packed[:, O_CN + KB + 512 : O_CN + KB + 1024])
            nc.sync.dma_start(mi_sb[:, KB + 1024 : KB + 1536],
                              packed[:, O_CN + KB + 1024 : O_CN + KB + 1536])
            nc.scalar.dma_start(mi_sb[:, KB + 1536 :],
                                packed[:, O_CN + KB + 1536 :])

            cn_sb = cpool.tile([P, KB], f32)
            nc.vector.tensor_copy(cn_sb[:], mi_sb[:, 0:KB])
            gb16 = cpool.tile([P, KB, BS], f16)

            _kbw = K_ATOMS + BS
            MIW = KB  # Minv starts after cneg inside mi_sb

            outr = out.rearrange("(mb p) n -> p mb n", p=P)

            # persistent PSUM: 4 banks, one tile per output chunk
            ps = [ppool.tile([P, BS], f32, name=f"ps{m}") for m in range(KB)]

            w_cur = [None] * KB

            # ---- iteration 1: G = Ht^T @ Yt ----
            for m in range(KB):
                for kb in range(2):
                    nc.tensor.matmul(
                        ps[m][:],
                        lhsT=hy_sb[:, kb * _kbw + m * P : kb * _kbw + (m + 1) * P],
                        rhs=hy_sb[:, kb * _kbw + K_ATOMS : (kb + 1) * _kbw],
                        start=(kb == 0),
                        stop=(kb == 1),
                    )
                if niter == 1:
                    xm = opool.tile([P, BS], f16, tag="x", name=f"x1{m}")
                    nc.scalar.activation(xm[:], ps[m][:], Act.Copy)
                    (nc.sync if m % 2 == 0 else nc.scalar).dma_start(
                        outr[:, m, :], xm[:]
                    )
                    continue
                wm = wpool.tile([P, BS], f16, tag="w", name=f"w1_{m}")
                nc.scalar.activation(wm[:], ps[m][:], Act.Abs, bias=nb[:, 0:1])
                w_cur[m] = wm
            if niter >= 2:
                # Gb = ps + cn: chunks 0,1 on DVE (tensor_scalar add), 2,3 on
                # ACT after the Abs ops - halves the iter-1 ACT chain
                def _g(m):
                    if m < 2:
                        nc.vector.tensor_scalar(gb16[:, m, :], ps[m][:],
                                                cn_sb[:, m : m + 1], None,
                                                Alu.add)
                    else:
                        nc.scalar.activation(gb16[:, m, :], ps[m][:],
                                             Act.Identity,
                                             bias=cn_sb[:, m : m + 1])

                def _f(m):
                    if niter == 2:
                        nc.vector.tensor_copy(ps[m][:], gb16[:, m, :])
                    else:
                        # S1 in place: ps <- min(ps, t) + Gb
                        nc.vector.scalar_tensor_tensor(
                            ps[m][:], ps[m][:], THR, gb16[:, m, :],
                            Alu.min, Alu.add,
                        )

                _g(0); _f(0); _g(1); _f(1)
                _g(2); _g(3); _f(2); _f(3)

            # ---- iterations 2..niter ----
            for it in range(2, niter + 1):
                last = it == niter
                neww = [None] * KB
                ORD = [(0, 0), (0, 1), (0, 2), (1, 0), (1, 1), (0, 3),
                       (1, 2), (1, 3), (2, 0), (2, 1), (2, 2), (2, 3),
                       (3, 0), (3, 1), (3, 2), (3, 3)]
                if last:
                    ORD = ORD[:-4]  # chunk 3 emitted as half groups below
                for m, kb in ORD:
                    nc.tensor.matmul(
                        ps[m][:],
                        lhsT=mi_sb[:, MIW + kb * K_ATOMS + m * P : MIW + kb * K_ATOMS + (m + 1) * P],
                        rhs=w_cur[kb][:],
                        start=False,
                        stop=(kb == KB - 1),
                    )
                    if kb != KB - 1:
                        continue
                    if last:
                        xm = opool.tile([P, BS], f16, tag="x", name=f"x{m}")
                        nc.scalar.activation(xm[:], ps[m][:], Act.Copy)
                        (nc.sync if m % 2 == 0 else nc.scalar).dma_start(
                            outr[:, m, :], xm[:]
                        )
                        continue
                    wm = wpool.tile([P, BS], f16, tag="w", name=f"w{it}_{m}")
                    nc.scalar.activation(wm[:], ps[m][:], Act.Abs, bias=nb[:, 0:1])
                    neww[m] = wm
                    if it == niter - 1:
                        # final accumulation target is Gb, not S'
                        nc.vector.tensor_copy(ps[m][:], gb16[:, m, :])
                    else:
                        nc.vector.scalar_tensor_tensor(
                            ps[m][:], ps[m][:], THR, gb16[:, m, :],
                            Alu.min, Alu.add,
                        )
                if last:
                    # chunk 3 in column-half groups: the first half's copy
                    # and 64KB transfer start while the second half computes
                    m = KB - 1
                    H = BS // 2
                    xm = opool.tile([P, BS], f16, tag="x", name=f"x{m}")
                    for h in range(2):
                        cs = slice(h * H, (h + 1) * H)
                        for kb in range(KB):
                            nc.tensor.matmul(
                                ps[m][:, cs],
                                lhsT=mi_sb[:, MIW + kb * K_ATOMS + m * P : MIW + kb * K_ATOMS + (m + 1) * P],
                                rhs=w_cur[kb][:, cs],
                                start=False,
                                stop=(kb == KB - 1),
                            )
                        nc.scalar.activation(xm[:, cs], ps[m][:, cs], Act.Copy)
                        (nc.sync if h == 0 else nc.scalar).dma_start(
                            outr[:, m, cs], xm[:, cs]
                        )
                else:
                    w_cur = neww

    nc.finalize()
    return nc


def _get_nc(niter):
    if niter not in _NC_CACHE:
        _NC_CACHE[niter] = _build(niter)
    return _NC_CACHE[niter]


def _prep_in_maps(Y, A):
    """Host precompute of the A-derived (voxel-independent) factor matrices,
    in float64: the inverse replaces the reference's Cholesky solve. Shards Y
    over voxels (transposed) and packs all device inputs into one
    pre-transposed [128, NPACK] fp16 array so every DMA descriptor is a
    multi-KB contiguous run."""
    A64 = A.astype(np.float64)
    LHS = A64.T @ A64 + RHO * np.eye(K_ATOMS)
    Minv = np.linalg.inv(LHS)
    Minv = (Minv + Minv.T) / 2
    Hm = A64 @ Minv  # [M, K]
    rsum = Minv.sum(axis=1)

    Ht = Hm.astype(np.float16)  # [M, K], M = 2*P exactly
    htp = Ht.reshape(2, P, K_ATOMS).transpose(1, 0, 2)  # [P, 2, K]
    Mi = Minv.astype(np.float16)
    mip = Mi.reshape(KB, P, K_ATOMS).transpose(1, 0, 2).reshape(P, KB * K_ATOMS)
    cneg = (-THR * rsum).astype(np.float16).reshape(KB, P).T  # [P, KB]
    fixed = np.concatenate([cneg, mip], axis=1)  # [P, KB + KB*K]

    in_maps = []
    for c in range(N_CORES):
        Yt = Y[c * BS : (c + 1) * BS, :].T.astype(np.float16)  # [M, BS]
        ytp = Yt.reshape(2, P, BS).transpose(1, 0, 2)  # [P, 2, BS]
        hy = np.concatenate([htp, ytp], axis=2).reshape(P, 2 * (K_ATOMS + BS))
        pk = np.ascontiguousarray(np.concatenate([hy, fixed], axis=1))
        in_maps.append({"packed": pk})
    return in_maps


def kernel(Y, A, max_iter):
    from concourse.bass_utils import run_bass_kernel_spmd

    Y = np.ascontiguousarray(np.asarray(Y, dtype=np.float32))
    A = np.ascontiguousarray(np.asarray(A, dtype=np.float32))
    niter = int(max_iter)
    assert Y.shape == (B_VOX, M_MEAS) and A.shape == (M_MEAS, K_ATOMS)
    if niter < 1:
        # zero-length scan returns the zero initial state
        return np.zeros((B_VOX, K_ATOMS), np.float32)

    in_maps = _prep_in_maps(Y, A)
    nc = _get_nc(niter)
    res = run_bass_kernel_spmd(nc, in_maps, core_ids=list(range(N_CORES)))

    outp = np.empty((B_VOX, K_ATOMS), np.float32)
    for c in range(N_CORES):
        outp[c * BS : (c + 1) * BS] = res.results[c]["out"].T.astype(np.float32)
    return outp


# revision 24
# speedup vs baseline: 1.0168x; 1.0168x over previous
"""AMICO ADMM solver on 8 Trainium2 NeuronCores.

Problem: X = argmin ||Y^T - A x||^2 + lam*||x||_1 s.t. x >= 0, solved with
max_iter ADMM steps (rho=1, lam=0.1) exactly as in the reference scan.

Algebraic reduction (tracking only v = x + u):
    v_1 = G                      with G  = Minv @ A^T @ Y^T
    for i = 2..N:
        w   = |v - t|            (t = lam/rho)
        S   = min(v, t) + Gb     (Gb = G - t * Minv @ 1)
        v'  = Minv @ w + S
    output x_N = Minv @ w_{N-1} + Gb

since z = relu(v - t), u' = v - z = min(v, t), and z - u' = |v - t| - t.

Sharding: data-parallel over voxels (B=4096 -> 512 per core); A-derived
matrices (Minv, Ht) replicated; no cross-core communication.

Implementation notes (measured on silicon):
 - All matmul operands are fp16 (fp8/bf16 lose too much accuracy over 30
   non-converged iterations; verified by simulation).
 - In-place S-fold: after each iteration's 4-matmul accumulation group
   fills PSUM bank m with v, ACT reads it for w = |v - t| and then DVE
   rewrites the bank IN PLACE with S' = min(v, t) + Gb.  The next
   iteration's matmuls accumulate onto S' with start=False - the PSUM
   has_written bits persist from the previous matmul group (only
   first_mm=1 clears them), so no identity matmuls are needed.  This
   cuts PE work from 18 to 16 matmuls/iteration and removes the
   separate fp16 v materialization from the DVE.
 - PE warm-up: ~8 dummy matmuls on a scratch PSUM bank issue during the
   input-DMA wait so the HAM clock gate reaches 2.4 GHz before the real
   matmuls start.
 - Input DMAs issue in parallel from the sync, scalar(ACT) and gpsimd
   queues; output DMAs are split across sync/gpsimd, per chunk, so the
   final transfers overlap the last iteration's compute.
"""

import numpy as np

B_VOX = 4096
M_MEAS = 256
K_ATOMS = 512
P = 128
N_CORES = 8
BS = B_VOX // N_CORES  # 512 voxels per core
KB = K_ATOMS // P  # 4 chunks of the contraction/output dim
LAM = 0.1
RHO = 1.0
THR = LAM / RHO

_NC_CACHE = {}

# packed layout offsets (fp16 elements per partition row)
O_HT0 = 0
O_YT0 = 512
O_HY1 = 1024            # Ht1 | Yt1
O_CN = 2048             # cneg [KB]
O_MI = 2052             # Minv, KB chunks of 512
NPACK = O_MI + KB * K_ATOMS  # 4100


def _build(niter):
    import concourse.mybir as mybir
    import concourse.tile as tile
    from concourse import bacc

    f32 = mybir.dt.float32
    f16 = mybir.dt.float16
    Alu = mybir.AluOpType
    Act = mybir.ActivationFunctionType

    nc = bacc.Bacc(None, target_bir_lowering=False)
    packed = nc.declare_dram_parameter("packed", [P, NPACK], f16, isOutput=False)
    out = nc.declare_dram_parameter("out", [K_ATOMS, BS], f16, isOutput=True)

    with tile.TileContext(nc) as tc:
        with (
            tc.tile_pool(name="const", bufs=1) as cpool,
            tc.tile_pool(name="w", bufs=8) as wpool,
            tc.tile_pool(name="o", bufs=4) as opool,
            tc.tile_pool(name="psum", bufs=1, space="PSUM") as ppool,
            tc.tile_pool(name="pwarm", bufs=1, space="PSUM") as ppwarm,
        ):
            # ---- PE warm-up: dummy matmuls on uninitialized scratch ----
            # (no data deps, so they issue right after the preamble barrier
            # and keep the HAM activity window busy during the DMA wait)
            sc_w = cpool.tile([P, P], f16)
            sc_r = cpool.tile([P, BS], f16)
            pwarm = ppwarm.tile([P, BS], f32)
            nc.vector.memset(sc_w[:], 0.0)
            nc.vector.memset(sc_r[:], 0.0)
            for _ in range(8):
                nc.tensor.matmul(pwarm[:], lhsT=sc_w[:], rhs=sc_r[:],
                                 start=True, stop=True)

            nb = cpool.tile([P, 1], f32)
            nc.vector.memset(nb[:], -THR)

            # ---- input DMAs: parallel issue across sync/scalar/gpsimd ----
            hy_sb = cpool.tile([P, 2 * (K_ATOMS + BS)], f16)
            mi_sb = cpool.tile([P, KB + KB * K_ATOMS], f16)  # cneg + Minv
            nc.sync.dma_start(hy_sb[:, 0:512], packed[:, O_HT0:O_YT0])
            nc.scalar.dma_start(hy_sb[:, 512:1024], packed[:, O_YT0:O_HY1])
            nc.sync.dma_start(hy_sb[:, 1024:1536], packed[:, O_HY1 : O_HY1 + 512])
            nc.scalar.dma_start(hy_sb[:, 1536:2048],
                                packed[:, O_HY1 + 512 : O_CN])
            nc.sync.dma_start(mi_sb[:, 0 : KB + 512],
                              packed[:, O_CN : O_CN + KB + 512])
            nc.scalar.dma_start(mi_sb[:, KB + 512 : KB + 1024],
                                packed[:, O_CN + KB + 512 : O_CN + KB + 1024])
            nc.sync.dma_start(mi_sb[:, KB + 1024 : KB + 1536],
                              packed[:, O_CN + KB + 1024 : O_CN + KB + 1536])
            nc.scalar.dma_start(mi_sb[:, KB + 1536 :],
                                packed[:, O_CN + KB + 1536 :])

            cn_sb = cpool.tile([P, KB], f32)
            nc.vector.tensor_copy(cn_sb[:], mi_sb[:, 0:KB])
            gb16 = cpool.tile([P, KB, BS], f16)

            _kbw = K_ATOMS + BS
            MIW = KB  # Minv starts after cneg inside mi_sb

            outr = out.rearrange("(mb p) n -> p mb n", p=P)

            # persistent PSUM: 4 banks, one tile per output chunk
            ps = [ppool.tile([P, BS], f32, name=f"ps{m}") for m in range(KB)]

            w_cur = [None] * KB

            # ---- iteration 1: G = Ht^T @ Yt ----
            for m in range(KB):
                for kb in range(2):
                    nc.tensor.matmul(
                        ps[m][:],
                        lhsT=hy_sb[:, kb * _kbw + m * P : kb * _kbw + (m + 1) * P],
                        rhs=hy_sb[:, kb * _kbw + K_ATOMS : (kb + 1) * _kbw],
                        start=(kb == 0),
                        stop=(kb == 1),
                    )
                if niter == 1:
                    xm = opool.tile([P, BS], f16, tag="x", name=f"x1{m}")
                    nc.scalar.activation(xm[:], ps[m][:], Act.Copy)
                    (nc.sync if m % 2 == 0 else nc.scalar).dma_start(
                        outr[:, m, :], xm[:]
                    )
                    continue
                wm = wpool.tile([P, BS], f16, tag="w", name=f"w1_{m}")
                nc.scalar.activation(wm[:], ps[m][:], Act.Abs, bias=nb[:, 0:1])
                w_cur[m] = wm
            if niter >= 2:
                # Gb = ps + cn: chunks 0,1 on DVE (tensor_scalar add), 2,3 on
                # ACT after the Abs ops - halves the iter-1 ACT chain
                def _g(m):
                    if m < 2:
                        nc.vector.tensor_scalar(gb16[:, m, :], ps[m][:],
                                                cn_sb[:, m : m + 1], None,
                                                Alu.add)
                    else:
                        nc.scalar.activation(gb16[:, m, :], ps[m][:],
                                             Act.Identity,
                                             bias=cn_sb[:, m : m + 1])

                def _f(m):
                    if niter == 2:
                        nc.vector.tensor_copy(ps[m][:], gb16[:, m, :])
                    else:
                        # S1 in place: ps <- min(ps, t) + Gb
                        nc.vector.scalar_tensor_tensor(
                            ps[m][:], ps[m][:], THR, gb16[:, m, :],
                            Alu.min, Alu.add,
                        )

                _g(0); _f(0); _g(1); _f(1)
                _g(2); _g(3); _f(2); _f(3)

            # ---- iterations 2..niter ----
            for it in range(2, niter + 1):
                last = it == niter
                neww = [None] * KB
                ORD = [(0, 0), (0, 1), (0, 2), (1, 0), (1, 1), (0, 3),
                       (1, 2), (1, 3), (2, 0), (2, 1), (2, 2), (2, 3),
                       (3, 0), (3, 1), (3, 2), (3, 3)]
                for m, kb in ORD:
                    nc.tensor.matmul(
                        ps[m][:],
                        lhsT=mi_sb[:, MIW + kb * K_ATOMS + m * P : MIW + kb * K_ATOMS + (m + 1) * P],
                        rhs=w_cur[kb][:],
                        start=False,
                        stop=(kb == KB - 1),
                    )
                    if kb != KB - 1:
                        continue
                    if last:
                        xm = opool.tile([P, BS], f16, tag="x", name=f"x{m}")
                        if m == KB - 1:
                            H = BS // 2
                            nc.scalar.activation(xm[:, 0:H], ps[m][:, 0:H],
                                                 Act.Copy)
                            nc.sync.dma_start(outr[:, m, 0:H], xm[:, 0:H])
                            nc.scalar.activation(xm[:, H:], ps[m][:, H:],
                                                 Act.Copy)
                            nc.scalar.dma_start(outr[:, m, H:], xm[:, H:])
                        else:
                            nc.scalar.activation(xm[:], ps[m][:], Act.Copy)
                            (nc.sync if m % 2 == 0 else nc.scalar).dma_start(
                                outr[:, m, :], xm[:]
                            )
                        continue
                    wm = wpool.tile([P, BS], f16, tag="w", name=f"w{it}_{m}")
                    nc.scalar.activation(wm[:], ps[m][:], Act.Abs, bias=nb[:, 0:1])
                    neww[m] = wm
                    if it == niter - 1:
                        # final accumulation target is Gb, not S'
                        nc.vector.tensor_copy(ps[m][:], gb16[:, m, :])
                    else:
                        nc.vector.scalar_tensor_tensor(
                            ps[m][:], ps[m][:], THR, gb16[:, m, :],
                            Alu.min, Alu.add,
                        )
                if not last:
                    w_cur = neww

    nc.finalize()
    return nc


def _get_nc(niter):
    if niter not in _NC_CACHE:
        _NC_CACHE[niter] = _build(niter)
    return _NC_CACHE[niter]


def _prep_in_maps(Y, A):
    """Host precompute of the A-derived (voxel-independent) factor matrices,
    in float64: the inverse replaces the reference's Cholesky solve. Shards Y
    over voxels (transposed) and packs all device inputs into one
    pre-transposed [128, NPACK] fp16 array so every DMA descriptor is a
    multi-KB contiguous run."""
    A64 = A.astype(np.float64)
    LHS = A64.T @ A64 + RHO * np.eye(K_ATOMS)
    Minv = np.linalg.inv(LHS)
    Minv = (Minv + Minv.T) / 2
    Hm = A64 @ Minv  # [M, K]
    rsum = Minv.sum(axis=1)

    Ht = Hm.astype(np.float16)  # [M, K], M = 2*P exactly
    htp = Ht.reshape(2, P, K_ATOMS).transpose(1, 0, 2)  # [P, 2, K]
    Mi = Minv.astype(np.float16)
    mip = Mi.reshape(KB, P, K_ATOMS).transpose(1, 0, 2).reshape(P, KB * K_ATOMS)
    cneg = (-THR * rsum).astype(np.float16).reshape(KB, P).T  # [P, KB]
    fixed = np.concatenate([cneg, mip], axis=1)  # [P, KB + KB*K]

    in_maps = []
    for c in range(N_CORES):
        Yt = Y[c * BS : (c + 1) * BS, :].T.astype(np.float16)  # [M, BS]
        ytp = Yt.reshape(2, P, BS).transpose(1, 0, 2)  # [P, 2, BS]
        hy = np.concatenate([htp, ytp], axis=2).reshape(P, 2 * (K_ATOMS + BS))
        pk = np.ascontiguousarray(np.concatenate([hy, fixed], axis=1))
        in_maps.append({"packed": pk})
    return in_maps


def kernel(Y, A, max_iter):
    from concourse.bass_utils import run_bass_kernel_spmd

    Y = np.ascontiguousarray(np.asarray(Y, dtype=np.float32))
    A = np.ascontiguousarray(np.asarray(A, dtype=np.float32))
    niter = int(max_iter)
    assert Y.shape == (B_VOX, M_MEAS) and A.shape == (M_MEAS, K_ATOMS)
    if niter < 1:
        # zero-length scan returns the zero initial state
        return np.zeros((B_VOX, K_ATOMS), np.float32)

    in_maps = _prep_in_maps(Y, A)
    nc = _get_nc(niter)
    res = run_bass_kernel_spmd(nc, in_maps, core_ids=list(range(N_CORES)))

    outp = np.empty((B_VOX, K_ATOMS), np.float32)
    for c in range(N_CORES):
        outp[c * BS : (c + 1) * BS] = res.results[c]["out"].T.astype(np.float32)
    return outp
